# revision 1
# baseline (speedup 1.0000x reference)
"""CollisionLoss kernel for 8 Trainium2 NeuronCores.

Data-parallel over batch: 512 trajectories split 64-per-core; each core
computes partial penalty sums, the host combines in float64 and divides by
B*N.

Per core, batches run 8 at a time (one per 16-partition group). Each batch's
200x200 ESDF map is held fp32 in SBUF as M row-decimated tables: group
partition 16g+m holds rows {m, m+M, ...} (H/M rows, F = (H/M)*W elements,
sized under the IndirectCopy ucode's 8K-element window). Rows ix and ix+1 of
a bilinear stencil live in tables (ix mod M) and ((ix+1) mod M) at the same
local index k = (ix div M)*W + iy, except ix mod M == M-1 which needs k+W in
table 0 -- so TWO GPSIMD indirect_copy calls with shared index streams k and
min(k+W, F-2) (inner size 2 fetches the iy/iy+1 pair) deliver all four
corners for every point. Which gathered stream is "row ix" vs "row ix+1" is
selected per point with is_equal masks on q = ix mod M.

Point prep (clip, floor, fracs, weights, in-range mask) runs on DVE/ACT over
all 128 partitions. Gathered pairs are redistributed from the M table
partitions back to the wrapped 128-partition layout by on-chip DMAs.
penalty = relu(3-10v)^2 and its reduction run on ACT (activation accum_out).
Out-of-range points get zeroed weights (v=0 -> penalty 9) plus a 160*(1-z)
correction -> 169 total, matching dists=-1 in the reference.

This walrus build rejects instructions carrying >1 sync wait; a post-pass
moves extra waits onto same-engine NoOps.
"""
import sys

sys.path.insert(0, "/opt/trn_rl_repo")

import numpy as np

import bass_rust
import concourse.bass as bass
import concourse.mybir as mybir
from concourse.tile import TileContext
from concourse import bass_utils
from concourse.bass_utils import run_bass_kernel_spmd

B, N, H, W = 512, 8192, 200, 200
NCORES = 8
BPC = B // NCORES            # 64 batches per core
GPB = 8                      # batches per round
ROUNDS = BPC // GPB          # 8
SW = N // 16                 # 512 point-slots per partition per batch
M = 10                       # row-decimation ways
RPT = H // M                 # rows per table (40)
F = RPT * W                  # 8000 elements per table
GK = 512                     # indices per indirect_copy call
HP = 2                       # sigma-buffer chunks per round
HN = N // HP                 # 4096 points per chunk
NSLOT = 2 * ROUNDS

F32 = mybir.dt.float32
U16 = mybir.dt.uint16
AF = mybir.ActivationFunctionType
OP = mybir.AluOpType

PROFILE = False
LAST_EXEC_NS = None

bass_utils.upload_artifacts = lambda tmpdir: "file://" + str(tmpdir)


def _split_multiwaits(nc):
    for fn in nc.m.functions:
        for bb in fn.blocks:
            insts = bb.instructions
            out = []
            changed = False
            for inst in insts:
                si = inst.sync_info
                waits = list(si.on_wait) if si is not None else []
                if len(waits) > 1:
                    for k, w in enumerate(waits[:-1]):
                        nop = bass_rust.InstNoOp(
                            name=f"{inst.name}-w{k}", engine=inst.engine)
                        nop.sync_info = bass_rust.SyncInfo(
                            on_wait=[w], on_update=[])
                        out.append(nop)
                    inst.sync_info = bass_rust.SyncInfo(
                        on_wait=[waits[-1]], on_update=list(si.on_update))
                    changed = True
                out.append(inst)
            if changed:
                bb.instructions = out


def _build():
    nc = bass.Bass()
    for v in (3.0,):
        t = nc.alloc_sbuf_tensor(f"const-float32-{v}", [128, 1], F32)
        nc.gpsimd.memset(t.ap(), v)
        nc.const_aps.aps[(F32, v)] = t.ap()
    nc.all_engine_barrier()
    ops_in = nc.dram_tensor("opState", [BPC, N * 2], F32, kind="ExternalInput")
    env_in = nc.dram_tensor("envs", [BPC, H * W], F32, kind="ExternalInput")
    part_out = nc.dram_tensor("partials", [128, NSLOT], F32,
                              kind="ExternalOutput")

    OS_t = nc.alloc_sbuf_tensor("osig", [128, N, 2], F32)
    G1P = [nc.alloc_sbuf_tensor(f"g1p{m}", [128, SW, 2], F32).ap()
           for m in range(M)]
    G2P = nc.alloc_sbuf_tensor("g2p", [128, SW, 2], F32).ap()

    with TileContext(nc) as tc:
        with (
            tc.tile_pool(name="tbl", bufs=1) as tblp,
            tc.tile_pool(name="pts", bufs=2) as ptsp,
            tc.tile_pool(name="prep", bufs=1) as prp,
            tc.tile_pool(name="widx", bufs=2) as wip,
            tc.tile_pool(name="gout", bufs=1) as gop,
            tc.tile_pool(name="comb", bufs=1) as cbp,
            tc.tile_pool(name="acc", bufs=1) as accp,
        ):
            table = tblp.tile([128, F], F32)
            t3d = table[:].rearrange("p (n d) -> p n d", d=2)  # [128,4000,2]
            partials = accp.tile([128, NSLOT], F32)

            env_ap = env_in.ap()
            ops_ap = ops_in.ap()

            for r in range(ROUNDS):
                b0 = r * GPB
                src = env_ap[b0:b0 + GPB, :].rearrange(
                    "g (h c) -> g h c", c=W)
                for m in range(M):
                    nc.sync.dma_start(out=table[m:128:16, :],
                                      in_=src[:, m:H:M, :])

                P = ptsp.tile([128, 2 * SW], F32)
                nc.sync.dma_start(
                    out=P[:],
                    in_=ops_ap[b0:b0 + GPB, :].rearrange(
                        "g (p f) -> (g p) f", p=16))

                # ---- prep: T = clip(x*10,-99,99)+99 = scaled coord - 0.5
                T = prp.tile([128, 2 * SW], F32)
                nc.vector.tensor_scalar(out=T[:], in0=P[:], scalar1=10.0,
                                        scalar2=-99.0, op0=OP.mult, op1=OP.max)
                nc.vector.tensor_scalar(out=T[:], in0=T[:], scalar1=99.0,
                                        scalar2=99.0, op0=OP.min, op1=OP.add)
                IC = prp.tile([128, 2 * SW], U16)    # floor of scaled coord
                nc.scalar.activation(out=IC[:], in_=T[:], func=AF.Copy)
                ITF = prp.tile([128, 2 * SW], F32)
                nc.scalar.activation(out=ITF[:], in_=IC[:], func=AF.Copy)
                ITFv = ITF[:].rearrange("p (s t) -> p s t", t=2)
                ITFX, ITFY = ITFv[:, :, 0], ITFv[:, :, 1]
                DXF = prp.tile([128, 2 * SW], F32)   # frac - 0.5
                nc.vector.tensor_tensor(out=DXF[:], in0=T[:], in1=ITF[:],
                                        op=OP.subtract)
                DXFv = DXF[:].rearrange("p (s t) -> p s t", t=2)
                DXFX, DXFY = DXFv[:, :, 0], DXFv[:, :, 1]

                # h = ix // M via round(ix/M - (M-1)/(2M)); q = ix - M*h
                HC = prp.tile([128, SW], U16)
                nc.scalar.activation(out=HC[:], in_=ITFX, func=AF.Copy,
                                     scale=1.0 / M, bias=-(M - 1) / (2.0 * M))
                HF = prp.tile([128, SW], F32)
                nc.scalar.activation(out=HF[:], in_=HC[:], func=AF.Copy)
                Q = prp.tile([128, SW], F32)
                nc.vector.tensor_scalar(out=Q[:], in0=HF[:],
                                        scalar1=-float(M), scalar2=None,
                                        op0=OP.mult)
                nc.vector.tensor_tensor(out=Q[:], in0=Q[:], in1=ITFX,
                                        op=OP.add)
                KF = prp.tile([128, SW], F32)        # k = h*W + iy
                nc.vector.tensor_scalar(out=KF[:], in0=HF[:],
                                        scalar1=float(W), scalar2=None,
                                        op0=OP.mult)
                nc.vector.tensor_tensor(out=KF[:], in0=KF[:], in1=ITFY,
                                        op=OP.add)
                IA = wip.tile([128, SW], U16)
                nc.scalar.activation(out=IA[:], in_=KF[:], func=AF.Copy)
                IB = wip.tile([128, SW], U16)
                nc.vector.tensor_scalar(out=IB[:], in0=KF[:],
                                        scalar1=float(W),
                                        scalar2=float(F - 2), op0=OP.add,
                                        op1=OP.min)

                # in-range mask z
                AB = prp.tile([128, 2 * SW], F32)
                nc.scalar.activation(out=AB[:], in_=P[:], func=AF.Abs)
                ABv = AB[:].rearrange("p (s t) -> p s t", t=2)
                ZM = prp.tile([128, SW], F32)
                nc.vector.tensor_tensor(out=ZM[:], in0=ABv[:, :, 0],
                                        in1=ABv[:, :, 1], op=OP.max)
                Z = prp.tile([128, SW], F32)
                nc.vector.tensor_scalar(out=Z[:], in0=ZM[:], scalar1=9.9,
                                        scalar2=None, op0=OP.is_le)
                CD = prp.tile([128, SW], F32)
                nc.scalar.activation(
                    out=CD[:], in_=Z[:], func=AF.Copy, scale=-160.0,
                    bias=160.0,
                    accum_out=partials[:, ROUNDS + r:ROUNDS + r + 1])

                # x/y fracs and row weights ax0 = (1-dx)z, ax1 = dx*z
                DXE = prp.tile([128, SW], F32)
                nc.scalar.activation(out=DXE[:], in_=DXFX, func=AF.Copy,
                                     bias=0.5)
                DYE = prp.tile([128, SW], F32)
                nc.scalar.activation(out=DYE[:], in_=DXFY, func=AF.Copy,
                                     bias=0.5)
                DYC = prp.tile([128, SW], F32)
                nc.scalar.activation(out=DYC[:], in_=DXFY, func=AF.Copy,
                                     scale=-1.0, bias=0.5)
                AX1 = prp.tile([128, SW], F32)
                nc.vector.tensor_tensor(out=AX1[:], in0=DXE[:], in1=Z[:],
                                        op=OP.mult)
                AX0 = prp.tile([128, SW], F32)
                nc.vector.tensor_tensor(out=AX0[:], in0=Z[:], in1=AX1[:],
                                        op=OP.subtract)
                # y-weight planes: A0E = ax0*(1-dy), A0O = ax0*dy, ...
                A0E = prp.tile([128, SW], F32)
                nc.vector.tensor_tensor(out=A0E[:], in0=AX0[:], in1=DYC[:],
                                        op=OP.mult)
                A0O = prp.tile([128, SW], F32)
                nc.vector.tensor_tensor(out=A0O[:], in0=AX0[:], in1=DYE[:],
                                        op=OP.mult)
                A1E = prp.tile([128, SW], F32)
                nc.vector.tensor_tensor(out=A1E[:], in0=AX1[:], in1=DYC[:],
                                        op=OP.mult)
                A1O = prp.tile([128, SW], F32)
                nc.vector.tensor_tensor(out=A1O[:], in0=AX1[:], in1=DYE[:],
                                        op=OP.mult)

                # ---- gather + redistribute (half-round sigma buffers)
                O1 = OS_t.ap()
                for c in range(N // GK):
                    i0, i1 = c * (GK // 16), (c + 1) * (GK // 16)
                    nc.gpsimd.indirect_copy(
                        O1[:, c * GK:(c + 1) * GK, :], t3d,
                        IA[:, i0:i1], True)
                for m in range(M):
                    for qq in range(16):
                        nc.sync.dma_start(
                            out=G1P[m][qq:qq + 113:16, :, :],
                            in_=O1[m:m + 113:16, :, :].rearrange(
                                "g (s u) c -> g s (u c)",
                                u=16)[:, :, 2 * qq:2 * qq + 2])
                O2 = OS_t.ap()
                for c in range(N // GK):
                    i0, i1 = c * (GK // 16), (c + 1) * (GK // 16)
                    nc.gpsimd.indirect_copy(
                        O2[:, c * GK:(c + 1) * GK, :], t3d,
                        IB[:, i0:i1], True)
                for qq in range(16):
                    nc.sync.dma_start(
                        out=G2P[qq:qq + 113:16, :, :],
                        in_=O2[0:113:16, :, :].rearrange(
                            "g (s u) c -> g s (u c)",
                            u=16)[:, :, 2 * qq:2 * qq + 2])

                # ---- select rows by q and combine
                # R0* = sum_m [q==m]*G1*[m]            (row ix planes)
                # R1* = sum_{m<M-1} [q==m]*G1*[m+1] + [q==M-1]*G2*
                R0E = cbp.tile([128, SW], F32, tag="r0e")
                R0O = cbp.tile([128, SW], F32, tag="r0o")
                R1E = cbp.tile([128, SW], F32, tag="r1e")
                R1O = cbp.tile([128, SW], F32, tag="r1o")
                TMP = cbp.tile([128, SW], F32, tag="tmp")
                for m in range(M):
                    EQ = cbp.tile([128, SW], F32, tag="eq")
                    nc.vector.tensor_scalar(out=EQ[:], in0=Q[:],
                                            scalar1=float(m), scalar2=None,
                                            op0=OP.is_equal)
                    se = (G1P[m + 1] if m < M - 1 else G2P)[:, :, 0]
                    so = (G1P[m + 1] if m < M - 1 else G2P)[:, :, 1]
                    for dst, g in ((R0E, G1P[m][:, :, 0]),
                                   (R0O, G1P[m][:, :, 1]),
                                   (R1E, se), (R1O, so)):
                        if m == 0:
                            nc.vector.tensor_tensor(out=dst[:], in0=EQ[:],
                                                    in1=g, op=OP.mult)
                        else:
                            nc.vector.tensor_tensor(out=TMP[:], in0=EQ[:],
                                                    in1=g, op=OP.mult)
                            nc.vector.tensor_tensor(out=dst[:], in0=dst[:],
                                                    in1=TMP[:], op=OP.add)

                U = cbp.tile([128, SW], F32, tag="u")
                nc.vector.tensor_tensor(out=U[:], in0=R0E[:], in1=A0E[:],
                                        op=OP.mult)
                UT = cbp.tile([128, SW], F32, tag="ut")
                for g, a in ((R0O, A0O), (R1E, A1E), (R1O, A1O)):
                    nc.vector.tensor_tensor(out=UT[:], in0=g[:], in1=a[:],
                                            op=OP.mult)
                    nc.vector.tensor_tensor(out=U[:], in0=U[:], in1=UT[:],
                                            op=OP.add)
                PS = U
                RT = cbp.tile([128, SW], F32, tag="rt")
                nc.scalar.activation(out=RT[:], in_=PS[:], func=AF.Relu,
                                     scale=-10.0, bias=3.0)
                SQ = cbp.tile([128, SW], F32, tag="sq")
                nc.scalar.activation(out=SQ[:], in_=RT[:], func=AF.Square,
                                     accum_out=partials[:, r:r + 1])

            nc.sync.dma_start(out=part_out.ap()[:], in_=partials[:])

    _split_multiwaits(nc)
    return nc


_CACHE = {}


def kernel(opState, envs):
    global LAST_EXEC_NS
    if "nc" not in _CACHE:
        _CACHE["nc"] = _build()
    nc = _CACHE["nc"]

    opState = np.ascontiguousarray(opState, dtype=np.float32)
    envs = np.ascontiguousarray(envs, dtype=np.float32)
    envs2 = envs.reshape(B, H * W)
    ops2 = opState.reshape(B, N * 2)

    in_maps = []
    for c in range(NCORES):
        sl = slice(c * BPC, (c + 1) * BPC)
        in_maps.append({
            "opState": np.ascontiguousarray(ops2[sl]),
            "envs": np.ascontiguousarray(envs2[sl]),
        })

    res = run_bass_kernel_spmd(nc, in_maps, core_ids=list(range(NCORES)),
                               trace=PROFILE)
    if res.exec_time_ns is not None:
        LAST_EXEC_NS = res.exec_time_ns

    total = np.float64(0.0)
    for r in res.results:
        total += r["partials"].astype(np.float64).sum()
    return np.float32(total / (B * N))


if __name__ == "__main__":
    d = np.load("/root/problem/work/ref_cache.npz")
    out = kernel(d["opState"], d["envs"])
    exp = float(d["out"])
    print("actual:", float(out), "expected:", exp,
          "rel:", abs(float(out) - exp) / abs(exp))



# revision 2
# speedup vs baseline: 3.2308x; 3.2308x over previous
"""CollisionLoss kernel v2 for 8 Trainium2 NeuronCores.

Data-parallel over batch: 512 trajectories split 64-per-core; each core
computes partial penalty sums; host combines and divides by B*N.

Tables: partition p = 16g + u permanently holds ONE fp16 plane: the
row-parity plane of batch bg = g*8 + u//2 with parity par = u%2. A plane
stores OVERLAPPED column pairs of its 100 rows:
   plane[k = m*200 + iy] = (E[2m+par, iy], E[2m+par, iy+1])
so one gather index fetches both column neighbours; rows ix and ix+1
live in the two parity planes at k1 = (ix//2)*200 + iy and
k2 = ((ix+1)//2)*200 + iy.

Round r: group g gathers batch g*8+r. Prep partition p holds points
[64p, 64p+64) of EVERY group's batch (free position 64g + delta), so the
idx int16 tensor is dense under the XBAR DMA transpose: storing
k_sigma(g, delta) at column 128*(delta//16) + 16g + delta%16 makes the
[32, 512]-slab transpose out[j, 32b+a] = in(a, 128b+j) deliver index
j = 16g + delta%16 -- exactly the gather's per-group wrapped partition.
A small per-partition strided copy then packs the gather slot order
s = 4p + delta//16 (= stream slot (64p+delta)//16). Two ap_gather calls
per sub-batch produce the column-pair streams for rows ix / ix+1 on the
two plane partitions of each group; a DRAM round-trip with contiguous
DMAs redistributes the 4 streams (2 rows x 2 parities) back to the prep
layout; DVE selects by row parity, blends bilinearly, applies the
in-range mask; ACT accumulates relu(3-10v)^2 plus a 160*(1-z)
out-of-range correction into per-round partial columns.

This walrus build rejects instructions carrying >1 sync wait; a post-pass
moves extra waits onto same-engine NoOps. InstAPGather must be lowered
via codegen_inst_isa_subclasses, and the ap_gather GPSIMD ucode library
loaded explicitly.
"""
import sys

sys.path.insert(0, "/opt/trn_rl_repo")

import numpy as np

import bass_rust
import concourse.bass as bass
import concourse.mybir as mybir
from concourse.tile import TileContext
from concourse import bass_utils, library_config
from concourse.bass_utils import run_bass_kernel_spmd

B, N, H, W = 512, 8192, 200, 200
NCORES = 8
BPC = B // NCORES            # 64 batches per core
ROUNDS = 8
NE = 20000                   # overlapped pairs per plane
NSB = 4                      # gather sub-batches per round
GIDX = N // NSB              # 2048 idx per gather instr

F32 = mybir.dt.float32
F16 = mybir.dt.float16
I16 = mybir.dt.int16
U16 = mybir.dt.uint16
AF = mybir.ActivationFunctionType
OP = mybir.AluOpType

PROFILE = False
LAST_EXEC_NS = None

bass_utils.upload_artifacts = lambda tmpdir: "file://" + str(tmpdir)


def _split_multiwaits(nc):
    for fn in nc.m.functions:
        for bb in fn.blocks:
            insts = bb.instructions
            out = []
            changed = False
            for inst in insts:
                si = inst.sync_info
                waits = list(si.on_wait) if si is not None else []
                if len(waits) > 1:
                    for k, w in enumerate(waits[:-1]):
                        nop = bass_rust.InstNoOp(
                            name=f"{inst.name}-w{k}", engine=inst.engine)
                        nop.sync_info = bass_rust.SyncInfo(
                            on_wait=[w], on_update=[])
                        out.append(nop)
                    inst.sync_info = bass_rust.SyncInfo(
                        on_wait=[waits[-1]], on_update=list(si.on_update))
                    changed = True
                out.append(inst)
            if changed:
                bb.instructions = out


def _build():
    nc = bass.Bass()
    for v in (3.0,):
        t = nc.alloc_sbuf_tensor(f"const-float32-{v}", [128, 1], F32)
        nc.gpsimd.memset(t.ap(), v)
        nc.const_aps.aps[(F32, v)] = t.ap()
    nc.gpsimd.load_library(library_config.ap_gather)
    nc.all_engine_barrier()
    ops_in = nc.dram_tensor("opState", [BPC, N * 2], F32, kind="ExternalInput")
    env_in = nc.dram_tensor("envs", [BPC, H * W], F32, kind="ExternalInput")
    part_out = nc.dram_tensor("partials", [128, 2 * ROUNDS], F32,
                              kind="ExternalOutput")
    # DRAM scratch for the value round-trip, double-buffered over rounds.
    # scr[buf][sigma][g][res] = one 8192-pair fp16 stream (16384 elems).
    scr = nc.dram_tensor("scr", [2, 2, 8, 2, N * 2], F16, kind="Internal")

    with TileContext(nc) as tc:
        with tc.tile_pool(name="tbl", bufs=1) as tblp:
            PL = tblp.tile([128, NE, 2], F16)
            with tc.tile_pool(name="rb", bufs=1) as rbp:
                RB = rbp.tile([128, NE], F32)
                env_v = env_in.ap().rearrange(
                    "b (hh par w) -> b hh par w", par=2, w=W)
                for par in range(2):
                    nc.sync.dma_start(
                        out=RB[par:128:2, :].rearrange(
                            "b (hh w) -> b hh w", w=W),
                        in_=env_v[:, :, par, :])
                nc.vector.tensor_copy(out=PL[:, :, 0], in_=RB[:])
                nc.vector.tensor_copy(out=PL[:, 0:NE - 1, 1],
                                      in_=RB[:, 1:NE])
                nc.vector.memset(PL[:, NE - 1:NE, 1], 0.0)

            # ops: partition p <- for each g: points [64p, 64p+64) of
            # batch g*8+r, at free position g*128 + delta*2 + d
            ops_v = ops_in.ap().rearrange(
                "(g rr) (p c) -> p rr g c", rr=ROUNDS, p=128)

            with (
                tc.tile_pool(name="pts", bufs=2) as ptsp,
                tc.tile_pool(name="prep", bufs=1) as prp,
                tc.tile_pool(name="xs", bufs=2) as xsp,
                tc.tile_pool(name="tw", bufs=1) as twp,
                tc.tile_pool(name="idxw", bufs=1) as iwp,
                tc.tile_pool(name="g", bufs=1) as gp,
                tc.tile_pool(name="vt", bufs=2) as vtp,
                tc.tile_pool(name="comb", bufs=1) as cbp,
                tc.tile_pool(name="acc", bufs=1) as accp,
            ):
                IDXW = iwp.tile([128, ROUNDS * 1024], I16)
                partials = accp.tile([128, 2 * ROUNDS], F32)

                for r in range(ROUNDS):
                    # ---- load points ----
                    P = ptsp.tile([128, 1024], F32, tag="p")
                    nc.sync.dma_start(
                        out=P[:].rearrange("p (g c) -> p g c", g=8),
                        in_=ops_v[:, r, :, :])

                    # ---- prep: T = clip(x*10,-99,99)+99 ----
                    T = prp.tile([128, 1024], F32, tag="t")
                    nc.vector.tensor_scalar(out=T[:], in0=P[:], scalar1=10.0,
                                            scalar2=-99.0, op0=OP.mult,
                                            op1=OP.max)
                    nc.vector.tensor_scalar(out=T[:], in0=T[:], scalar1=99.0,
                                            scalar2=99.0, op0=OP.min,
                                            op1=OP.add)
                    IC = prp.tile([128, 1024], U16, tag="ic")
                    nc.scalar.activation(out=IC[:], in_=T[:], func=AF.Copy)
                    ITF = prp.tile([128, 1024], F32, tag="itf")
                    nc.scalar.activation(out=ITF[:], in_=IC[:], func=AF.Copy)
                    ITFv = ITF[:].rearrange("p (s t) -> p s t", t=2)
                    IX, IY = ITFv[:, :, 0], ITFv[:, :, 1]
                    # frac-0.5 in place of T (T dead after this)
                    D = T
                    nc.vector.tensor_tensor(out=D[:], in0=T[:], in1=ITF[:],
                                            op=OP.subtract)
                    Dv = D[:].rearrange("p (s t) -> p s t", t=2)
                    DXE = ptsp.tile([128, 512], F32, tag="dxe")
                    nc.scalar.activation(out=DXE[:], in_=Dv[:, :, 0],
                                         func=AF.Copy, bias=0.5)
                    DYE = ptsp.tile([128, 512], F32, tag="dye")
                    nc.scalar.activation(out=DYE[:], in_=Dv[:, :, 1],
                                         func=AF.Copy, bias=0.5)

                    AB = prp.tile([128, 1024], F32, tag="ab")
                    nc.scalar.activation(out=AB[:], in_=P[:], func=AF.Abs)
                    ABv = AB[:].rearrange("p (s t) -> p s t", t=2)
                    ZM = prp.tile([128, 512], F32, tag="zm")
                    nc.vector.tensor_tensor(out=ZM[:], in0=ABv[:, :, 0],
                                            in1=ABv[:, :, 1], op=OP.max)
                    Z = ptsp.tile([128, 512], F32, tag="z")
                    nc.vector.tensor_scalar(out=Z[:], in0=ZM[:], scalar1=9.9,
                                            scalar2=None, op0=OP.is_le)

                    HC = prp.tile([128, 512], U16, tag="hc")
                    nc.scalar.activation(out=HC[:], in_=IX, func=AF.Copy,
                                         scale=0.5, bias=-0.25)
                    HF = prp.tile([128, 512], F32, tag="hf")
                    nc.scalar.activation(out=HF[:], in_=HC[:], func=AF.Copy)
                    PAR = ptsp.tile([128, 512], F32, tag="par")
                    nc.vector.tensor_scalar(out=PAR[:], in0=HF[:],
                                            scalar1=-2.0, scalar2=None,
                                            op0=OP.mult)
                    nc.vector.tensor_tensor(out=PAR[:], in0=PAR[:], in1=IX,
                                            op=OP.add)
                    K1 = prp.tile([128, 512], F32, tag="k1")
                    nc.vector.tensor_scalar(out=K1[:], in0=HF[:],
                                            scalar1=float(W), scalar2=None,
                                            op0=OP.mult)
                    nc.vector.tensor_tensor(out=K1[:], in0=K1[:], in1=IY,
                                            op=OP.add)
                    K2 = prp.tile([128, 512], F32, tag="k2")
                    nc.vector.tensor_scalar(out=K2[:], in0=PAR[:],
                                            scalar1=float(W), scalar2=None,
                                            op0=OP.mult)
                    nc.vector.tensor_tensor(out=K2[:], in0=K2[:], in1=K1[:],
                                            op=OP.add)

                    # ---- casts into XBAR-dense idxC ----
                    # idxC[p, 512*sg + 128*(delta//16) + 16*g + delta%16]
                    #   = k_sg(g, 64p + delta)
                    IDXC = xsp.tile([128, 1024], I16, tag="idxc")
                    for sg, K in ((0, K1), (1, K2)):
                        Kv = K[:].rearrange("p (g d4 w) -> p g d4 w",
                                            d4=4, w=16)
                        Cv = IDXC[:, 512 * sg:512 * (sg + 1)].rearrange(
                            "p (d4 g w) -> p d4 g w", g=8, w=16)
                        for dd in range(4):
                            nc.scalar.activation(out=Cv[:, dd, :, :],
                                                 in_=Kv[:, :, dd, :],
                                                 func=AF.Copy)

                    # ---- XBAR: out[j, 128q + 32b + a] = idxC[32q+a,
                    #      512sg + 128b + j] ----
                    TW0 = twp.tile([128, 512], I16, tag="tw0")
                    TW1 = twp.tile([128, 512], I16, tag="tw1")
                    for sg, TWt in ((0, TW0), (1, TW1)):
                        for q in range(4):
                            eng = nc.scalar if q % 2 == 0 else nc.sync
                            eng.dma_start(
                                out=TWt[:, 128 * q:128 * (q + 1)].rearrange(
                                    "j (b a) -> j b a", a=32),
                                in_=IDXC[32 * q:32 * (q + 1),
                                         512 * sg:512 * (sg + 1)],
                                transpose=True)

                    # ---- pack gather slot order ----
                    # IDXW[j, 1024r + 512sg + 128q + 4a + b]
                    #   = TW[j, 128q + 32b + a]
                    for sg, TWt in ((0, TW0), (1, TW1)):
                        for q in range(4):
                            iv = TWt[:, 128 * q:128 * (q + 1)].rearrange(
                                "p (b a) -> p b a", a=32)
                            ov = IDXW[:, r * 1024 + 512 * sg + 128 * q:
                                      r * 1024 + 512 * sg + 128 * (q + 1)
                                      ].rearrange("p (a b) -> p b a", b=4)
                            if q % 2 == 0:
                                nc.vector.tensor_copy(out=ov, in_=iv)
                            else:
                                nc.scalar.activation(out=ov, in_=iv,
                                                     func=AF.Copy)

                    # ---- gathers + DMA-A ----
                    for sb in range(NSB):
                        for sg in range(2):
                            G = gp.tile([128, GIDX, 2], F16, tag=f"g{sg}")
                            nc.gpsimd.ap_gather(
                                G[:], PL[:],
                                IDXW[:, r * 1024 + sg * 512 + sb * 128:
                                     r * 1024 + sg * 512 + (sb + 1) * 128],
                                channels=128, num_elems=NE, d=2,
                                num_idxs=GIDX)
                            for res in range(2):
                                nc.sync.dma_start(
                                    out=scr.ap()[r % 2, sg, :, res,
                                                 sb * 2 * GIDX:
                                                 (sb + 1) * 2 * GIDX],
                                    in_=G[2 * r + res:2 * r + res + 113:16,
                                          :, :].rearrange(
                                        "g n d -> g (n d)"))

                    # ---- DMA-B: back to prep layout ----
                    VT = []
                    for sg in range(2):
                        for res in range(2):
                            V = vtp.tile([128, 8, 64, 2], F16,
                                         tag=f"v{sg}{res}")
                            nc.sync.dma_start(
                                out=V[:].rearrange("p g n d -> p g (n d)"),
                                in_=scr.ap()[r % 2, sg, :, res, :].rearrange(
                                    "g (p c) -> p g c", p=128))
                            VT.append(V)
                    VA, VB, VC, VD = VT

                    def lane(Vt, e):
                        return Vt[:].rearrange("p g n d -> p (g n) d")[:, :, e]

                    # row ix = sel(par: 0->A, 1->B); row ix+1 = sel(par:
                    # 0->D, 1->C)
                    def sel(mfrom, mto, tag):
                        TT = cbp.tile([128, 512], F32, tag="tt")
                        nc.vector.tensor_tensor(out=TT[:], in0=mto,
                                                in1=mfrom, op=OP.subtract)
                        nc.vector.tensor_tensor(out=TT[:], in0=TT[:],
                                                in1=PAR[:], op=OP.mult)
                        o = cbp.tile([128, 512], F32, tag=tag)
                        nc.vector.tensor_tensor(out=o[:], in0=TT[:],
                                                in1=mfrom, op=OP.add)
                        return o

                    R0c0 = sel(lane(VA, 0), lane(VB, 0), "r0c0")
                    R0c1 = sel(lane(VA, 1), lane(VB, 1), "r0c1")
                    R1c0 = sel(lane(VD, 0), lane(VC, 0), "r1c0")
                    R1c1 = sel(lane(VD, 1), lane(VC, 1), "r1c1")

                    def blend(c0, c1, wgt, tag):
                        TT = cbp.tile([128, 512], F32, tag="tt")
                        nc.vector.tensor_tensor(out=TT[:], in0=c1[:],
                                                in1=c0[:], op=OP.subtract)
                        nc.vector.tensor_tensor(out=TT[:], in0=TT[:],
                                                in1=wgt[:], op=OP.mult)
                        o = cbp.tile([128, 512], F32, tag=tag)
                        nc.vector.tensor_tensor(out=o[:], in0=TT[:],
                                                in1=c0[:], op=OP.add)
                        return o

                    V0 = blend(R0c0, R0c1, DYE, "v0")
                    V1 = blend(R1c0, R1c1, DYE, "v1")
                    VV = blend(V0, V1, DXE, "vv")
                    VZ = cbp.tile([128, 512], F32, tag="vz")
                    nc.vector.tensor_tensor(out=VZ[:], in0=VV[:], in1=Z[:],
                                            op=OP.mult)

                    RT = cbp.tile([128, 512], F32, tag="rt")
                    nc.scalar.activation(out=RT[:], in_=VZ[:], func=AF.Relu,
                                         scale=-10.0, bias=3.0)
                    SQ = cbp.tile([128, 512], F32, tag="sq")
                    nc.scalar.activation(out=SQ[:], in_=RT[:], func=AF.Square,
                                         accum_out=partials[:, r:r + 1])
                    CD = cbp.tile([128, 512], F32, tag="cd")
                    nc.scalar.activation(
                        out=CD[:], in_=Z[:], func=AF.Copy, scale=-160.0,
                        bias=160.0,
                        accum_out=partials[:, ROUNDS + r:ROUNDS + r + 1])

                nc.sync.dma_start(out=part_out.ap()[:], in_=partials[:])

    mybir.codegen_inst_isa_subclasses(nc)
    _split_multiwaits(nc)
    return nc


_CACHE = {}


def kernel(opState, envs):
    global LAST_EXEC_NS
    if "nc" not in _CACHE:
        _CACHE["nc"] = _build()
    nc = _CACHE["nc"]

    opState = np.ascontiguousarray(opState, dtype=np.float32)
    envs = np.ascontiguousarray(envs, dtype=np.float32)
    envs2 = envs.reshape(B, H * W)
    ops2 = opState.reshape(B, N * 2)

    in_maps = []
    for c in range(NCORES):
        sl = slice(c * BPC, (c + 1) * BPC)
        in_maps.append({
            "opState": np.ascontiguousarray(ops2[sl]),
            "envs": np.ascontiguousarray(envs2[sl]),
        })

    res = run_bass_kernel_spmd(nc, in_maps, core_ids=list(range(NCORES)),
                               trace=PROFILE)
    if res.exec_time_ns is not None:
        LAST_EXEC_NS = res.exec_time_ns

    total = np.float64(0.0)
    for r in res.results:
        total += r["partials"].astype(np.float64).sum()
    return np.float32(total / (B * N))


if __name__ == "__main__":
    d = np.load("/root/problem/work/ref_cache.npz")
    out = kernel(d["opState"], d["envs"])
    exp = float(d["out"])
    print("actual:", float(out), "expected:", exp,
          "rel:", abs(float(out) - exp) / abs(exp))


# revision 3
# speedup vs baseline: 5.0334x; 1.5580x over previous
"""CollisionLoss kernel v3 for 8 Trainium2 NeuronCores.

Data-parallel over batch: 512 trajectories split 64-per-core; each core
computes partial penalty sums; host combines and divides by B*N.

ap_gather costs ~27ns/index regardless of d (SBUF read-command bound),
so v3 fetches the whole 2x2 bilinear patch with ONE uint8 d=4 index per
point (v2 used two fp16 d=2 indices). Tables: partition p = 16g + u
holds the HALF-plane of batch g*8 + u//2 with half = u%2: uint8
quantized (2/255 step) overlapped patches
   plane[k] = (E[r,c], E[r,c+1], E[r+1,c], E[r+1,c+1]),
half 0 from rows [0,101) (k = ix*200 + iy, ix in [0,99)), half 1 from
rows [99,200) (k = (ix-99)*200 + iy). The dequant scale folds into the
penalty activation: relu(3 - (20/255)*v_raw)^2.

Round r: group g gathers batch g*8+r. Prep partition p holds points
[64p, 64p+64) of EVERY group's batch (free position 64g + delta); the
idx int16 tensor is dense under the XBAR DMA transpose: storing
k(g, delta) at column 128*(delta//16) + 16g + delta%16 makes the
[32, 512]-slab transpose out[j, 32b+a] = in(a, 128b+j) deliver index
j = 16g + delta%16 -- the gather's per-group wrapped partition. A small
per-partition strided copy packs gather slot order s = 4p + delta//16.
A DRAM round-trip with contiguous DMAs redistributes the two half-plane
streams back to the prep layout; DVE selects by half, blends
bilinearly, applies the in-range mask; ACT accumulates the penalty plus
a 160*(1-z) out-of-range correction into per-round partial columns.

This walrus build rejects instructions carrying >1 sync wait; a post-pass
moves extra waits onto same-engine NoOps. InstAPGather must be lowered
via codegen_inst_isa_subclasses, and the ap_gather GPSIMD ucode library
loaded explicitly.
"""
import sys

sys.path.insert(0, "/opt/trn_rl_repo")

import numpy as np

import bass_rust
import concourse.bass as bass
import concourse.mybir as mybir
from concourse.tile import TileContext
from concourse import bass_utils, library_config
from concourse.bass_utils import run_bass_kernel_spmd

B, N, H, W = 512, 8192, 200, 200
NCORES = 8
BPC = B // NCORES            # 64 batches per core
ROUNDS = 8
NE = 20000                   # patches per half-plane
NSRC = 20200                 # source values per half-plane (101 rows)
NSB = 4                      # gather sub-batches per round
GIDX = N // NSB              # 2048 idx per gather instr
QS = 127.5                   # uint8 quantization scale

F32 = mybir.dt.float32
F16 = mybir.dt.float16
I16 = mybir.dt.int16
U16 = mybir.dt.uint16
U8 = mybir.dt.uint8
AF = mybir.ActivationFunctionType
OP = mybir.AluOpType

PROFILE = False
LAST_EXEC_NS = None

bass_utils.upload_artifacts = lambda tmpdir: "file://" + str(tmpdir)


def _split_multiwaits(nc):
    for fn in nc.m.functions:
        for bb in fn.blocks:
            insts = bb.instructions
            out = []
            changed = False
            for inst in insts:
                si = inst.sync_info
                waits = list(si.on_wait) if si is not None else []
                if len(waits) > 1:
                    for k, w in enumerate(waits[:-1]):
                        nop = bass_rust.InstNoOp(
                            name=f"{inst.name}-w{k}", engine=inst.engine)
                        nop.sync_info = bass_rust.SyncInfo(
                            on_wait=[w], on_update=[])
                        out.append(nop)
                    inst.sync_info = bass_rust.SyncInfo(
                        on_wait=[waits[-1]], on_update=list(si.on_update))
                    changed = True
                out.append(inst)
            if changed:
                bb.instructions = out


def _build():
    nc = bass.Bass()
    for v in (3.0,):
        t = nc.alloc_sbuf_tensor(f"const-float32-{v}", [128, 1], F32)
        nc.gpsimd.memset(t.ap(), v)
        nc.const_aps.aps[(F32, v)] = t.ap()
    nc.gpsimd.load_library(library_config.ap_gather)
    nc.all_engine_barrier()
    ops_in = nc.dram_tensor("opState", [BPC, N * 2], F32, kind="ExternalInput")
    env_in = nc.dram_tensor("envs", [BPC, H * W], F32, kind="ExternalInput")
    part_out = nc.dram_tensor("partials", [128, 2 * ROUNDS], F32,
                              kind="ExternalOutput")
    # DRAM scratch: scr[buf][g][half] = one 8192-patch u8 stream (32KB).
    scr = nc.dram_tensor("scr", [2, 8, 2, N * 4], U8, kind="Internal")

    with TileContext(nc) as tc:
        with tc.tile_pool(name="tbl", bufs=1) as tblp:
            PL = tblp.tile([128, NE, 4], U8)
            with tc.tile_pool(name="rb", bufs=1) as rbp:
                RB = rbp.tile([128, NSRC], F32)
                for half in range(2):
                    nc.sync.dma_start(
                        out=RB[half:128:2, :],
                        in_=env_in.ap()[:, 19800 * half:
                                        19800 * half + NSRC])
                Q = rbp.tile([128, NSRC], U8)
                nc.vector.tensor_scalar(out=Q[:], in0=RB[:], scalar1=QS,
                                        scalar2=None, op0=OP.mult)
                nc.vector.tensor_copy(out=PL[:, :, 0], in_=Q[:, 0:NE])
                nc.vector.tensor_copy(out=PL[:, :, 1], in_=Q[:, 1:NE + 1])
                nc.vector.tensor_copy(out=PL[:, :, 2], in_=Q[:, 200:NE + 200])
                nc.vector.tensor_copy(out=PL[:, 0:NE - 1, 3],
                                      in_=Q[:, 201:NE + 200])
                nc.vector.memset(PL[:, NE - 1:NE, 3], 0.0)

            # partition p <- for each g: points [64p, 64p+64) of batch
            # g*8+r at free position g*128 + delta*2 + d
            ops_v = ops_in.ap().rearrange(
                "(g rr) (p c) -> p rr g c", rr=ROUNDS, p=128)

            with (
                tc.tile_pool(name="pts", bufs=2) as ptsp,
                tc.tile_pool(name="prep", bufs=1) as prp,
                tc.tile_pool(name="xs", bufs=2) as xsp,
                tc.tile_pool(name="tw", bufs=2) as twp,
                tc.tile_pool(name="idxw", bufs=1) as iwp,
                tc.tile_pool(name="g", bufs=2) as gp,
                tc.tile_pool(name="vt", bufs=2) as vtp,
                tc.tile_pool(name="comb", bufs=1) as cbp,
                tc.tile_pool(name="acc", bufs=1) as accp,
            ):
                IDXW = iwp.tile([128, ROUNDS * 512], I16)
                partials = accp.tile([128, 2 * ROUNDS], F32)

                for r in range(ROUNDS):
                    # ---- load points ----
                    P = ptsp.tile([128, 1024], F32, tag="p")
                    nc.sync.dma_start(
                        out=P[:].rearrange("p (g c) -> p g c", g=8),
                        in_=ops_v[:, r, :, :])

                    # ---- prep ----
                    T = prp.tile([128, 1024], F32, tag="t")
                    nc.vector.tensor_scalar(out=T[:], in0=P[:], scalar1=10.0,
                                            scalar2=-99.0, op0=OP.mult,
                                            op1=OP.max)
                    nc.vector.tensor_scalar(out=T[:], in0=T[:], scalar1=99.0,
                                            scalar2=99.0, op0=OP.min,
                                            op1=OP.add)
                    IC = prp.tile([128, 1024], U16, tag="ic")
                    nc.scalar.activation(out=IC[:], in_=T[:], func=AF.Copy)
                    ITF = prp.tile([128, 1024], F32, tag="itf")
                    nc.scalar.activation(out=ITF[:], in_=IC[:], func=AF.Copy)
                    ITFv = ITF[:].rearrange("p (s t) -> p s t", t=2)
                    IX, IY = ITFv[:, :, 0], ITFv[:, :, 1]
                    # frac - 0.5 in place of T
                    D = T
                    nc.vector.tensor_tensor(out=D[:], in0=T[:], in1=ITF[:],
                                            op=OP.subtract)
                    Dv = D[:].rearrange("p (s t) -> p s t", t=2)
                    DXE = ptsp.tile([128, 512], F32, tag="dxe")
                    nc.scalar.activation(out=DXE[:], in_=Dv[:, :, 0],
                                         func=AF.Copy, bias=0.5)
                    DYE = ptsp.tile([128, 512], F32, tag="dye")
                    nc.scalar.activation(out=DYE[:], in_=Dv[:, :, 1],
                                         func=AF.Copy, bias=0.5)

                    AB = prp.tile([128, 1024], F32, tag="ab")
                    nc.scalar.activation(out=AB[:], in_=P[:], func=AF.Abs)
                    ABv = AB[:].rearrange("p (s t) -> p s t", t=2)
                    ZM = prp.tile([128, 512], F32, tag="zm")
                    nc.vector.tensor_tensor(out=ZM[:], in0=ABv[:, :, 0],
                                            in1=ABv[:, :, 1], op=OP.max)
                    Z = ptsp.tile([128, 512], F32, tag="z")
                    nc.vector.tensor_scalar(out=Z[:], in0=ZM[:], scalar1=9.9,
                                            scalar2=None, op0=OP.is_le)

                    # half select: HB = (ix >= 99)
                    HB = ptsp.tile([128, 512], F32, tag="hb")
                    nc.vector.tensor_scalar(out=HB[:], in0=IX, scalar1=98.5,
                                            scalar2=None, op0=OP.is_gt)
                    # k = ix*200 + iy - 19800*HB
                    K1 = prp.tile([128, 512], F32, tag="k1")
                    nc.vector.tensor_scalar(out=K1[:], in0=IX,
                                            scalar1=float(W), scalar2=None,
                                            op0=OP.mult)
                    nc.vector.tensor_tensor(out=K1[:], in0=K1[:], in1=IY,
                                            op=OP.add)
                    TMP = prp.tile([128, 512], F32, tag="tmp")
                    nc.vector.tensor_scalar(out=TMP[:], in0=HB[:],
                                            scalar1=-19800.0, scalar2=None,
                                            op0=OP.mult)
                    nc.vector.tensor_tensor(out=K1[:], in0=K1[:], in1=TMP[:],
                                            op=OP.add)

                    # ---- cast into XBAR-dense idxC ----
                    # idxC[p, 128*(delta//16) + 16*g + delta%16]
                    IDXC = xsp.tile([128, 512], I16, tag="idxc")
                    Kv = K1[:].rearrange("p (g d4 w) -> p g d4 w",
                                         d4=4, w=16)
                    Cv = IDXC[:].rearrange("p (d4 g w) -> p d4 g w",
                                           g=8, w=16)
                    for dd in range(4):
                        nc.scalar.activation(out=Cv[:, dd, :, :],
                                             in_=Kv[:, :, dd, :],
                                             func=AF.Copy)

                    # ---- XBAR: out[j, 128q + 32b + a] =
                    #      idxC[32q+a, 128b + j] ----
                    TW = twp.tile([128, 512], I16, tag="tw")
                    for q in range(4):
                        eng = nc.scalar if q % 2 == 0 else nc.sync
                        eng.dma_start(
                            out=TW[:, 128 * q:128 * (q + 1)].rearrange(
                                "j (b a) -> j b a", a=32),
                            in_=IDXC[32 * q:32 * (q + 1), :],
                            transpose=True)

                    # ---- pack gather slot order ----
                    # IDXW[j, 512r + 128q + 4a + b] = TW[j, 128q + 32b + a]
                    for q in range(4):
                        iv = TW[:, 128 * q:128 * (q + 1)].rearrange(
                            "p (b a) -> p b a", a=32)
                        ov = IDXW[:, r * 512 + 128 * q:
                                  r * 512 + 128 * (q + 1)].rearrange(
                            "p (a b) -> p b a", b=4)
                        if q % 2 == 0:
                            nc.vector.tensor_copy(out=ov, in_=iv)
                        else:
                            nc.scalar.activation(out=ov, in_=iv,
                                                 func=AF.Copy)

                    # ---- gathers + DMA-A ----
                    for sb in range(NSB):
                        G = gp.tile([128, GIDX, 4], U8, tag="g")
                        nc.gpsimd.ap_gather(
                            G[:], PL[:],
                            IDXW[:, r * 512 + sb * 128:
                                 r * 512 + (sb + 1) * 128],
                            channels=128, num_elems=NE, d=4,
                            num_idxs=GIDX)
                        for res in range(2):
                            nc.sync.dma_start(
                                out=scr.ap()[r % 2, :, res,
                                             sb * 4 * GIDX:
                                             (sb + 1) * 4 * GIDX],
                                in_=G[2 * r + res:2 * r + res + 113:16,
                                      :, :].rearrange("g n d -> g (n d)"))

                    # ---- DMA-B: back to prep layout ----
                    VT = []
                    for res in range(2):
                        V = vtp.tile([128, 8, 64, 4], U8, tag=f"v{res}")
                        nc.sync.dma_start(
                            out=V[:].rearrange("p g n d -> p g (n d)"),
                            in_=scr.ap()[r % 2, :, res, :].rearrange(
                                "g (p c) -> p g c", p=128))
                        VT.append(V)
                    VTOP, VBOT = VT

                    def lane(Vt, e):
                        return Vt[:].rearrange("p g n d -> p (g n) d")[:, :, e]

                    # patch = HB ? VBOT : VTOP, per lane
                    def sel(e, tag):
                        mfrom, mto = lane(VTOP, e), lane(VBOT, e)
                        TT = cbp.tile([128, 512], F32, tag="tt")
                        nc.vector.tensor_tensor(out=TT[:], in0=mto,
                                                in1=mfrom, op=OP.subtract)
                        nc.vector.tensor_tensor(out=TT[:], in0=TT[:],
                                                in1=HB[:], op=OP.mult)
                        o = cbp.tile([128, 512], F32, tag=tag)
                        nc.vector.tensor_tensor(out=o[:], in0=TT[:],
                                                in1=mfrom, op=OP.add)
                        return o

                    L0 = sel(0, "l0")
                    L1 = sel(1, "l1")
                    L2 = sel(2, "l2")
                    L3 = sel(3, "l3")

                    def blend(c0, c1, wgt, tag):
                        TT = cbp.tile([128, 512], F32, tag="tt")
                        nc.vector.tensor_tensor(out=TT[:], in0=c1[:],
                                                in1=c0[:], op=OP.subtract)
                        nc.vector.tensor_tensor(out=TT[:], in0=TT[:],
                                                in1=wgt[:], op=OP.mult)
                        o = cbp.tile([128, 512], F32, tag=tag)
                        nc.vector.tensor_tensor(out=o[:], in0=TT[:],
                                                in1=c0[:], op=OP.add)
                        return o

                    V0 = blend(L0, L1, DYE, "v0")
                    V1 = blend(L2, L3, DYE, "v1")
                    VV = blend(V0, V1, DXE, "vv")
                    VZ = cbp.tile([128, 512], F32, tag="vz")
                    nc.vector.tensor_tensor(out=VZ[:], in0=VV[:], in1=Z[:],
                                            op=OP.mult)

                    # penalty with folded dequant: relu(3 - (10/QS)*vz)^2
                    RT = cbp.tile([128, 512], F32, tag="rt")
                    nc.scalar.activation(out=RT[:], in_=VZ[:], func=AF.Relu,
                                         scale=-10.0 / QS, bias=3.0)
                    SQ = cbp.tile([128, 512], F32, tag="sq")
                    nc.scalar.activation(out=SQ[:], in_=RT[:], func=AF.Square,
                                         accum_out=partials[:, r:r + 1])
                    CD = cbp.tile([128, 512], F32, tag="cd")
                    nc.scalar.activation(
                        out=CD[:], in_=Z[:], func=AF.Copy, scale=-160.0,
                        bias=160.0,
                        accum_out=partials[:, ROUNDS + r:ROUNDS + r + 1])

                nc.sync.dma_start(out=part_out.ap()[:], in_=partials[:])

    mybir.codegen_inst_isa_subclasses(nc)
    _split_multiwaits(nc)
    return nc


_CACHE = {}


def kernel(opState, envs):
    global LAST_EXEC_NS
    if "nc" not in _CACHE:
        _CACHE["nc"] = _build()
    nc = _CACHE["nc"]

    opState = np.ascontiguousarray(opState, dtype=np.float32)
    envs = np.ascontiguousarray(envs, dtype=np.float32)
    envs2 = envs.reshape(B, H * W)
    ops2 = opState.reshape(B, N * 2)

    in_maps = []
    for c in range(NCORES):
        sl = slice(c * BPC, (c + 1) * BPC)
        in_maps.append({
            "opState": np.ascontiguousarray(ops2[sl]),
            "envs": np.ascontiguousarray(envs2[sl]),
        })

    res = run_bass_kernel_spmd(nc, in_maps, core_ids=list(range(NCORES)),
                               trace=PROFILE)
    if res.exec_time_ns is not None:
        LAST_EXEC_NS = res.exec_time_ns

    total = np.float64(0.0)
    for r in res.results:
        total += r["partials"].astype(np.float64).sum()
    return np.float32(total / (B * N))


if __name__ == "__main__":
    d = np.load("/root/problem/work/ref_cache.npz")
    out = kernel(d["opState"], d["envs"])
    exp = float(d["out"])
    print("actual:", float(out), "expected:", exp,
          "rel:", abs(float(out) - exp) / abs(exp))
